# revision 37
# baseline (speedup 1.0000x reference)
"""Trainium2 Bass kernel for nn_AttenCross (sparse_attention).

reference:
    scores = einsum('bqd,bkd->bqk', Q, D) / sqrt(H)
    scores = where(doc_mask==0, -9999, scores)
    attn   = softmax(scores, -1)
    out    = sum over k of (attn * sim), then sum over q -> (B, 1)

Strategy (8 cores, data-parallel over batch, 2 batches/core), v3:

Host-side prep (sharding/layout/encoding only, exact for any inputs):
~50% of doc positions are masked (doc_mask ~ Bernoulli(0.5)), and masked
columns contribute exactly nothing once handled by counting, so the doc
axis is COMPACTED host-side: keep only unmasked doc columns (of D and
sim), zero-pad to K_pad = ceil(max_b keff[b]/16)*16.  A padded column
of D is all-zero => its score is exactly 0 => exp is exactly 1, so
subtracting the pad count from the exp row-sum reproduces the exact
softmax denominator; padded sim columns are zero so they add nothing to
the numerator.  (No row-max subtraction: scores ~ N(0,1); softmax is
shift-invariant.)  All tensors are converted to fp16 host-side: PE runs
fp16 at full rate (1 col/cycle vs ~2.2 for f32r), DMA bytes halve, and
DVE 16-bit ops run in 2x/4x perf modes; fp16's 10-bit mantissa keeps the
overall rel-err ~1e-3.

Device, per batch (per q-tile of 128 queries), ACT(exp)-paced pipeline:
  - PE: QK^T fp16 matmuls into PSUM chunks (1024 double-buffered +
    K_pad-1024 single-buffered; PSUM is 8 banks: 4 + 3 + 1 for the
    column-sum accumulator -- a full-width double-buffered score tile
    would not fit, which is what forces 2 ACTIVATEs per q-tile).
  - ACT: E = exp(scale*psum) -> fp16 SBUF, one ACTIVATE per chunk, with
    accum_out producing the per-chunk row-sums (den parts).
  - DVE: den-parts sum + cnt subtract + reciprocal -> w into column 0 of
    a rotating fp16 [128,128] tile; P = E * sim fp16 (2x mode), split at
    the chunk boundary so it starts right after chunk A's exp.
  - PE: column-sum matmuls with w as the stationary operand accumulate
    sum_q P[q,k]/den_q into a per-batch [128,512] PSUM bank; row 0 of
    that bank reduced (DVE) to the scalar batch output at the end.
    These trail the scores pipeline by one q-tile (software pipelining).

v3 pipeline changes over v2 (v2 measured 69.3us, v3 measured 65.7us;
the 32-ACTIVATE exp stream is gapless at its floor (~2.43us/q-tile =
1773ns of elems + 2x293ns instruction bubbles; PSUM's 8 banks make the
2-chunk/2-bubble structure unavoidable), so all v3 wins are at the
edges -- the fill before the first exp, the ramp, and the tail):
  - FILL (first exp 15.9us -> 12.5us): the v2 critical path serialized
    qt(256KB)+dtA(256KB)+dtB(276KB) on the sync HWDGE queue.  v3 runs
    exactly one ~2KB-row transfer per queue in parallel -- scalar: dt
    chunk A, sync: qt, gpsimd: dt chunk B + cnt -- finishing together
    at the measured floor (~11.6us = preamble 6.7 + issue 0.7 + shared
    ~115GB/s transfer + ~1-2us HBM write-receipt before the completion
    semaphore fires).  Finer pieces do NOT help: SDMA engines round-
    robin across queues at packet granularity (small-row queues get
    starved proportionally), completions serialize behind a straggler
    engine, and PSUM-pool deps are whole-tile so a partial chunk can't
    start its ACTIVATE early anyway.  Both sim streams queue BEHIND the
    critical fill transfers on their queues; batch 1's prefetch is
    emitted at tile 5 so it lands in sim-stream gaps.
  - WARM-UP: the ~4.5us of PE idle during the fill left the HAM clock
    gate at 1.2GHz into the first q-tiles (cold 512-wide matmul 630ns
    vs 379 warm) and v2 re-throttled mid-ramp.  10 dummy matmuls on an
    all-zero tile (gated only on its memset, parked on the PE queue
    ahead of the real work) keep the PE continuously busy from 7.5us
    so it un-throttles at ~11.5us and stays at 2.4GHz for the whole
    run (one HAM window in the trace).
  - DEFER=2 column-sums (one tile in v2): the colsum chain for tile t
    (sim DMA -> P=E*sim -> den/recip -> 9 matmuls) lands on the
    in-order PE queue two steps late, so the scores matmuls for tiles
    t+1/t+2 are never stuck behind it during the ramp (this was v2's
    hidden mid-ramp stall; one ~1.2us pipeline-fill transient remains).
  - TAIL: 256-wide colsum segments (same PE streaming cost, half the
    final [1,CSEG] reduce), per-batch out DMAs (batch 0's scalar goes
    out mid-kernel; only batch 1's [1,1] store + ~1us DMA receipt sit
    on the tail), and the P-mult piece that doesn't depend on the last
    ACTIVATE is emitted before the den chain so the strict-FIFO DVE
    queue doesn't serialize it into the tail.
Fixed overheads measured: ~6.7us NEFF preamble, ~8.1us teardown (every
engine zeroes its 51-semaphore slice + barriers) -- untouchable from
kernel code; steady-state ACT is the pacer at ~98% busy.
Output per core: [1, BPC] fp32; host stacks to [16, 1] fp32.
"""

import math

import numpy as np

import concourse.bacc as bacc
import concourse.tile as tile
import concourse.mybir as mybir
from concourse.bass_utils import run_bass_kernel_spmd

B, QL, DL, H = 16, 1024, 4096, 128
NCORES = 8
BPC = B // NCORES  # batches per core
QT_N = QL // 128  # 8 q-tiles per batch
SCALE = 1.0 / float(np.sqrt(H))
NWARM = 10  # PE warm-up matmuls during the DMA fill

f32 = mybir.dt.float32
f16 = mybir.dt.float16

_CACHED = {}


def _plan_chunks(k_pad):
    """Split the doc axis into PSUM-resident chunks: (offset_lo, offset_hi,
    tag, bufs).

    Expected path (k_pad <= 2560): chunk A [128,1024] double-buffered (4
    banks) + chunk B [128, k_pad-1024 <= 1536] single-buffered (<=3
    banks) + 1 bank for the column-sum accumulator = 8.  Generic
    fallback for larger k_pad: 1024-wide chunks cycling three
    single-buffered tags."""
    if k_pad <= 1024:
        return [(0, k_pad, "A", 2)]
    if k_pad <= 2560:
        return [(0, 1024, "A", 2), (1024, k_pad, "B", 1)]
    chunks = []
    off = 0
    i = 0
    while off < k_pad:
        w = min(1024, k_pad - off)
        chunks.append((off, off + w, "BCD"[i % 3], 1))
        off += w
        i += 1
    return chunks


def _build(k_pad):
    chunks = _plan_chunks(k_pad)

    nc = bacc.Bacc("TRN2", target_bir_lowering=False, debug=False)

    qtd = nc.dram_tensor("qt", [BPC, H, QL], f16, kind="ExternalInput")
    dtd = nc.dram_tensor("dt", [BPC, H, k_pad], f16, kind="ExternalInput")
    sd = nc.dram_tensor("s", [BPC, QL, k_pad], f16, kind="ExternalInput")
    ckd = nc.dram_tensor("ck", [BPC, 1], f32, kind="ExternalInput")
    outd = nc.dram_tensor("o", [1, BPC], f32, kind="ExternalOutput")

    with tile.TileContext(nc) as tc:
        with (
            tc.tile_pool(name="const", bufs=1) as const,
            tc.tile_pool(name="qtp", bufs=2) as qtp,
            tc.tile_pool(name="dtp", bufs=2) as dtp,
            tc.tile_pool(name="simp", bufs=4) as simp,
            tc.tile_pool(name="ep", bufs=3) as ep,
            tc.tile_pool(name="pp", bufs=3) as pp,
            tc.tile_pool(name="small", bufs=4) as small,
            tc.tile_pool(name="bsm", bufs=2) as bsm,
            tc.tile_pool(name="outp", bufs=1) as outp,
            tc.tile_pool(name="ps", bufs=1, space="PSUM") as psp,
            tc.tile_pool(name="pacc", bufs=1, space="PSUM") as pacc,
        ):
            # all-zero tile for the fill-phase HAM warm-up matmuls
            # (memset FIRST: it is the only thing the warm-ups wait on)
            wz = const.tile([128, 512], f16, tag="warmz", name="warmz")
            nc.vector.memset(wz, 0.0)
            # stationary w-tiles for the column-sum matmuls: col 0 = 1/den,
            # cols 1..127 stay zero forever (three rotating, since the
            # column-sums trail the scores pipeline by DEFER=2 steps)
            r128s = []
            for ri in range(3):
                r = const.tile([128, 128], f16, tag=f"r128_{ri}", name=f"r128_{ri}")
                nc.vector.memset(r, 0.0)
                r128s.append(r)

            outsb = outp.tile([1, BPC], f32, tag="outsb")

            import concourse.bass as _bass

            # column-sum segments: 256-wide (vs 512) costs no PE streaming
            # time (warm back-to-back gap ~= N/2.4 + NX) but halves the
            # final batch-epilogue reduce on the tail critical path
            CSEG = 256
            ncol = (k_pad + CSEG - 1) // CSEG
            nchunks = len(chunks)
            firstw = chunks[0][1]  # end of the first chunk
            state = {}

            # pad counts for BOTH batches in one broadcast DMA during the
            # fill (partition-broadcast, BPC values per partition row) so
            # batch 1's prefetch never needs the slow 128-descriptor
            # broadcast mid-kernel.
            cntk2 = bsm.tile([128, BPC], f32, tag="cntk", name="cntk")
            ck_ap = ckd.ap()[:, :]
            ck_bcast = _bass.AP(
                tensor=ck_ap.tensor,
                offset=ck_ap.offset,
                ap=[[0, 128], [1, BPC]],
            )

            def setup_batch(b, first):
                dt = dtp.tile([128, k_pad], f16, tag="dt", name=f"dt{b}")
                qt = qtp.tile([128, QL], f16, tag="qt", name=f"qt{b}")
                if first:
                    # Only sync/scalar (HWDGE) and gpsimd (SWDGE) can issue
                    # DMAs.  SDMA engines round-robin between queues at
                    # PACKET granularity (a queue with smaller rows is
                    # starved in proportion), and each transfer's
                    # completion semaphore lags its last packet by the
                    # ~1-2us HBM write-receipt.  PSUM-pool dependencies are
                    # whole-tile, so finer dt pieces cannot start the exp
                    # stream any earlier than the whole chunk anyway: the
                    # fill is exactly one ~2KB-row critical transfer per
                    # queue, all finishing together at the measured floor
                    # (first exp ~12.7us):
                    # scalar: dt chunk A, sync: qt, gpsimd: dt chunk B +
                    # the cnt broadcast.  The sim streams queue BEHIND
                    # these, so they never steal fill bandwidth.
                    nc.scalar.dma_start(
                        dt[:, :firstw], dtd.ap()[b][:, :firstw]
                    )
                    nc.sync.dma_start(qt, qtd.ap()[b])
                    if firstw < k_pad:
                        nc.gpsimd.dma_start(dt[:, firstw:], dtd.ap()[b][:, firstw:])
                    nc.gpsimd.dma_start(cntk2, ck_bcast)
                else:
                    # batch 1's prefetch (emitted mid-batch-0 so its
                    # transfers land between the sim streams, which the
                    # 4-deep sim prefetch absorbs): qt + first dt chunk on
                    # sync, dt tail on gpsimd.
                    nc.sync.dma_start(qt, qtd.ap()[b])
                    nc.sync.dma_start(dt[:, :firstw], dtd.ap()[b][:, :firstw])
                    if firstw < k_pad:
                        nc.gpsimd.dma_start(dt[:, firstw:], dtd.ap()[b][:, firstw:])
                state[b] = (qt, dt)

            setup_batch(0, True)

            # HAM warm-up: the PE sits idle for the ~4.5us DMA fill, which
            # leaves the clock gate at 1.2GHz into the first q-tiles.
            # Dummy matmuls on the all-zero tile (gated only on its
            # memset) keep the PE busy through the fill so it un-throttles
            # (~3.4us sustained) before the first real QK; they park on
            # the PE queue ahead of the real matmuls, ending ~when the
            # first dt piece lands.
            lo, hi, tag0, cbufs0 = chunks[0]
            if hi - lo >= 512:
                for wi in range(NWARM):
                    pscd = psp.tile(
                        [128, hi - lo], f32, tag=tag0, bufs=cbufs0,
                        name=f"pscw{wi}",
                    )
                    nc.tensor.matmul(
                        pscd[:, :512],
                        r128s[0],
                        wz,
                        start=True,
                        stop=True,
                        skip_group_check=True,
                    )

            NSTEP = BPC * QT_N
            # column-sums trail the scores pipeline by DEFER steps so the
            # next q-tiles' score matmuls never wait on the multiply chain
            # (sim DMA -> P -> den/recip -> colsum).  DEFER=2 keeps the
            # in-order PE queue from blocking on tile t's chain while tile
            # t+2's scores are due; measured best among DEFER 1/2/3 (the
            # Tile scheduler makes the final queue order either way).
            DEFER = 2
            pendings = []  # deferred column-sums: (b, t, acc, r128, p_t)

            for s in range(NSTEP + DEFER):
                if s < NSTEP:
                    b, t = divmod(s, QT_N)
                    if t == 5 and b + 1 < BPC:
                        setup_batch(b + 1, False)
                    qt, dt = state[b][0], state[b][1]
                    if t == 0:
                        acc_b = pacc.tile(
                            [128, CSEG], f32, tag="acc", name=f"acc{b}"
                        )
                        state[(b, "acc")] = acc_b
                    acc = state[(b, "acc")]

                    sim = simp.tile(
                        [128, k_pad], f16, tag="sim", name=f"sim{b}_{t}"
                    )
                    # alternate the sim streams across the sync HWDGE queue
                    # and the GpSimd SWDGE queue for DMA parallelism (even
                    # tiles behind qt on sync, odd tiles behind dt-B/cnt on
                    # gpsimd).
                    dma_eng = nc.sync if t % 2 == 0 else nc.gpsimd
                    dma_eng.dma_start(
                        sim, sd.ap()[b, t * 128 : (t + 1) * 128, :]
                    )
                    e_t = ep.tile([128, k_pad], f16, tag="E", name=f"e{b}_{t}")
                    den2 = small.tile(
                        [128, 8], f32, tag="den2", name=f"den2_{b}_{t}"
                    )
                    nparts = 0
                    for ci, (lo, hi, tag, cbufs) in enumerate(chunks):
                        psc = psp.tile(
                            [128, hi - lo], f32, tag=tag, bufs=cbufs,
                            name=f"psc{tag}",
                        )
                        for s0 in range(0, hi - lo, 512):
                            s1 = min(s0 + 512, hi - lo)
                            nc.tensor.matmul(
                                psc[:, s0:s1],
                                qt[:, t * 128 : (t + 1) * 128],
                                dt[:, lo + s0 : lo + s1],
                                start=True,
                                stop=True,
                            )
                        nc.scalar.activation(
                            out=e_t[:, lo:hi],
                            in_=psc,
                            func=mybir.ActivationFunctionType.Exp,
                            scale=SCALE,
                            accum_out=den2[:, nparts : nparts + 1],
                        )
                        nparts += 1

                if s == NSTEP and chunks[0][1] - chunks[0][0] >= 512:
                    # keep-warm: the PE idles ~5us between the last QK and
                    # the last column-sums, long enough for the HAM clock
                    # gate to re-throttle to 1.2GHz.  Two dummy matmuls
                    # gated on the last exp / multiply refresh the activity
                    # window mid-gap so the final column-sums run at 2.4GHz.
                    lo, hi, tag, cbufs = chunks[0]
                    for di, mv in enumerate((last_e, last_p)):
                        pscd = psp.tile(
                            [128, hi - lo], f32, tag=tag, bufs=cbufs,
                            name=f"pscwarm{di}",
                        )
                        nc.tensor.matmul(
                            pscd[:, :512],
                            r128s[1],
                            mv[:, :512],
                            start=True,
                            stop=True,
                            skip_group_check=True,
                        )

                # deferred column-sums, deprioritized so the Tile scheduler
                # keeps them behind the next q-tiles' score matmuls on the
                # in-order PE queue: the scores pipeline must never wait on
                # the DVE multiply chain (sim DMA -> P -> den/recip) that
                # feeds a column-sum.
                if len(pendings) > (DEFER - 1 if s < NSTEP else -1):
                    pb, pt, pacc_t, pr128, p_prev = pendings.pop(0)
                    for j in range(ncol):
                        s0 = j * CSEG
                        s1 = min(s0 + CSEG, k_pad)
                        nc.tensor.matmul(
                            pacc_t[:, : s1 - s0],
                            pr128,
                            p_prev[:, s0:s1],
                            start=(pt == 0 and j == 0),
                            stop=(pt == QT_N - 1 and j == ncol - 1),
                            skip_group_check=True,
                        )
                    if pt == QT_N - 1:
                        # batch epilogue: row 0 of acc = sum over q of
                        # P[q,k]/den_q; store each batch's scalar as soon
                        # as it is ready so only the last batch's [1,1]
                        # DMA sits on the tail.
                        nc.vector.reduce_sum(
                            outsb[0:1, pb : pb + 1],
                            pacc_t[0:1, :],
                            axis=mybir.AxisListType.X,
                        )
                        nc.sync.dma_start(
                            outd.ap()[:, pb : pb + 1],
                            outsb[0:1, pb : pb + 1],
                        )

                if s < NSTEP:
                    cntk = cntk2[:, b : b + 1]
                    # P in pieces, each starting right after the matching
                    # exp ACTIVATE.  All but the last piece are emitted
                    # BEFORE the den chain: the den chain waits on the LAST
                    # chunk's accumulator read, and the DVE queue is strict
                    # FIFO, so P work queued behind it would serialize into
                    # the tail after the final ACTIVATE.
                    p_t = pp.tile([128, k_pad], f16, tag="P", name=f"p{b}_{t}")
                    psplits = sorted(set([0, firstw, k_pad]))
                    for pi in range(len(psplits) - 2):
                        w0, w1 = psplits[pi], psplits[pi + 1]
                        nc.vector.tensor_tensor(
                            p_t[:, w0:w1], e_t[:, w0:w1], sim[:, w0:w1],
                            mybir.AluOpType.mult,
                        )
                    dent = small.tile([128, 1], f32, tag="dent", name="dent")
                    if nparts == 2:
                        # denA - cnt overlaps chunk B's ACTIVATE; only the
                        # tiny add + reciprocal remain on the tail chain
                        denta = small.tile(
                            [128, 1], f32, tag="denta", name="denta"
                        )
                        nc.vector.tensor_scalar(
                            denta, den2[:, 0:1], cntk, None,
                            mybir.AluOpType.subtract,
                        )
                        nc.vector.tensor_tensor(
                            dent, denta, den2[:, 1:2], mybir.AluOpType.add
                        )
                    else:
                        den = small.tile([128, 1], f32, tag="den", name="den")
                        nc.vector.reduce_sum(
                            den, den2[:, :nparts], axis=mybir.AxisListType.X
                        )
                        nc.vector.tensor_scalar(
                            dent, den, cntk, None, mybir.AluOpType.subtract
                        )
                    r128 = r128s[s % 3]
                    with nc.allow_low_precision(
                        reason="1/den in fp16 (11-bit mantissa) feeds the PE "
                        "column-sum; ~5e-4 relative, inside the error budget"
                    ):
                        nc.vector.reciprocal(r128[:, 0:1], dent)
                    w0, w1 = psplits[-2], psplits[-1]
                    nc.vector.tensor_tensor(
                        p_t[:, w0:w1], e_t[:, w0:w1], sim[:, w0:w1],
                        mybir.AluOpType.mult,
                    )
                    pendings.append((b, t, acc, r128, p_t))
                    last_e, last_p = e_t, p_t

    nc.compile()
    return nc


def kernel(**inputs: np.ndarray) -> np.ndarray:
    q = np.asarray(inputs["query_input"], dtype=np.float32)
    d = np.asarray(inputs["doc_input"], dtype=np.float32)
    s = np.asarray(inputs["sim_matrix"], dtype=np.float32)
    dm = np.asarray(inputs["doc_mask"]) != 0  # [B, DL]

    keff = dm.sum(axis=1).astype(np.int64)  # [B]
    k_pad = int(min(DL, max(128, math.ceil(int(keff.max()) / 16) * 16)))

    if k_pad not in _CACHED:
        _CACHED[k_pad] = _build(k_pad)
    nc = _CACHED[k_pad]

    qt = np.ascontiguousarray(np.swapaxes(q, 1, 2)).astype(np.float16)
    dtc = np.zeros((B, H, k_pad), dtype=np.float16)
    simc = np.zeros((B, QL, k_pad), dtype=np.float16)
    for b in range(B):
        idx = np.flatnonzero(dm[b])
        ke = idx.size
        dtc[b, :, :ke] = d[b, idx, :].T
        simc[b, :, :ke] = s[b][:, idx]
    ck = (k_pad - keff).astype(np.float32).reshape(B, 1)

    in_maps = []
    for c in range(NCORES):
        lo, hi = c * BPC, (c + 1) * BPC
        in_maps.append(
            {
                "qt": qt[lo:hi],
                "dt": dtc[lo:hi],
                "s": simc[lo:hi],
                "ck": ck[lo:hi],
            }
        )

    out = None
    for attempt in range(3):
        try:
            res = run_bass_kernel_spmd(nc, in_maps, core_ids=list(range(NCORES)))
            # materialize inside the retry: transient device wedges can
            # surface as late as the device->host copy
            out = np.concatenate(
                [
                    np.asarray(res.results[c]["o"]).reshape(BPC)
                    for c in range(NCORES)
                ],
                axis=0,
            )
            break
        except Exception:
            if attempt == 2:
                raise
    return out.reshape(B, 1).astype(np.float32)


# revision 39
# speedup vs baseline: 1.0227x; 1.0227x over previous
"""Trainium2 Bass kernel for nn_AttenCross (sparse_attention).

reference:
    scores = einsum('bqd,bkd->bqk', Q, D) / sqrt(H)
    scores = where(doc_mask==0, -9999, scores)
    attn   = softmax(scores, -1)
    out    = sum over k of (attn * sim), then sum over q -> (B, 1)

Strategy (8 cores, data-parallel over batch, 2 batches/core), v3:

Host-side prep (sharding/layout/encoding only, exact for any inputs):
~50% of doc positions are masked (doc_mask ~ Bernoulli(0.5)), and masked
columns contribute exactly nothing once handled by counting, so the doc
axis is COMPACTED host-side: keep only unmasked doc columns (of D and
sim), zero-pad to K_pad = ceil(max_b keff[b]/16)*16.  A padded column
of D is all-zero => its score is exactly 0 => exp is exactly 1, so
subtracting the pad count from the exp row-sum reproduces the exact
softmax denominator; padded sim columns are zero so they add nothing to
the numerator.  (No row-max subtraction: scores ~ N(0,1); softmax is
shift-invariant.)  All tensors are converted to fp16 host-side: PE runs
fp16 at full rate (1 col/cycle vs ~2.2 for f32r), DMA bytes halve, and
DVE 16-bit ops run in 2x/4x perf modes; fp16's 10-bit mantissa keeps the
overall rel-err ~1e-3.

Device, per batch (per q-tile of 128 queries), ACT(exp)-paced pipeline:
  - PE: QK^T fp16 matmuls into PSUM chunks (1024 double-buffered +
    K_pad-1024 single-buffered; PSUM is 8 banks: 4 + 3 + 1 for the
    column-sum accumulator -- a full-width double-buffered score tile
    would not fit, which is what forces 2 ACTIVATEs per q-tile).
  - ACT: E = exp(scale*psum) -> fp16 SBUF, one ACTIVATE per chunk, with
    accum_out producing the per-chunk row-sums (den parts).
  - DVE: den-parts sum + cnt subtract + reciprocal -> w into column 0 of
    a rotating fp16 [128,128] tile; P = E * sim fp16 (2x mode), split at
    the chunk boundary so it starts right after chunk A's exp.
  - PE: column-sum matmuls with w as the stationary operand accumulate
    sum_q P[q,k]/den_q into a per-batch [128,512] PSUM bank; row 0 of
    that bank reduced (DVE) to the scalar batch output at the end.
    These trail the scores pipeline by one q-tile (software pipelining).

v3 pipeline changes over v2 (v2 measured 69.3us, v3 measured 65.7us;
the 32-ACTIVATE exp stream is gapless at its floor (~2.43us/q-tile =
1773ns of elems + 2x293ns instruction bubbles; PSUM's 8 banks make the
2-chunk/2-bubble structure unavoidable), so all v3 wins are at the
edges -- the fill before the first exp, the ramp, and the tail):
  - FILL (first exp 15.9us -> 12.5us): the v2 critical path serialized
    qt(256KB)+dtA(256KB)+dtB(276KB) on the sync HWDGE queue.  v3 runs
    exactly one ~2KB-row transfer per queue in parallel -- scalar: dt
    chunk A, sync: qt, gpsimd: dt chunk B + cnt -- finishing together
    at the measured floor (~11.6us = preamble 6.7 + issue 0.7 + shared
    ~115GB/s transfer + ~1-2us HBM write-receipt before the completion
    semaphore fires).  Finer pieces do NOT help: SDMA engines round-
    robin across queues at packet granularity (small-row queues get
    starved proportionally), completions serialize behind a straggler
    engine, and PSUM-pool deps are whole-tile so a partial chunk can't
    start its ACTIVATE early anyway.  Both sim streams queue BEHIND the
    critical fill transfers on their queues; batch 1's prefetch is
    emitted at tile 5 so it lands in sim-stream gaps.
  - WARM-UP: the ~4.5us of PE idle during the fill left the HAM clock
    gate at 1.2GHz into the first q-tiles (cold 512-wide matmul 630ns
    vs 379 warm) and v2 re-throttled mid-ramp.  10 dummy matmuls on an
    all-zero tile (gated only on its memset, parked on the PE queue
    ahead of the real work) keep the PE continuously busy from 7.5us
    so it un-throttles at ~11.5us and stays at 2.4GHz for the whole
    run (one HAM window in the trace).
  - DEFER=2 column-sums (one tile in v2): the colsum chain for tile t
    (sim DMA -> P=E*sim -> den/recip -> 9 matmuls) lands on the
    in-order PE queue two steps late, so the scores matmuls for tiles
    t+1/t+2 are never stuck behind it during the ramp (this was v2's
    hidden mid-ramp stall; one ~1.2us pipeline-fill transient remains).
  - TAIL: 256-wide colsum segments (same PE streaming cost, half the
    final [1,CSEG] reduce), per-batch out DMAs (batch 0's scalar goes
    out mid-kernel; only batch 1's [1,1] store + ~1us DMA receipt sit
    on the tail), and the P-mult piece that doesn't depend on the last
    ACTIVATE is emitted before the den chain so the strict-FIFO DVE
    queue doesn't serialize it into the tail.
Fixed overheads measured: ~6.7us NEFF preamble, ~8.1us teardown (every
engine zeroes its 51-semaphore slice + barriers) -- untouchable from
kernel code; steady-state ACT is the pacer at ~98% busy.
Output per core: [1, BPC] fp32; host stacks to [16, 1] fp32.
"""

import math

import numpy as np

import concourse.bacc as bacc
import concourse.tile as tile
import concourse.mybir as mybir
from concourse.bass_utils import run_bass_kernel_spmd

B, QL, DL, H = 16, 1024, 4096, 128
NCORES = 8
BPC = B // NCORES  # batches per core
QT_N = QL // 128  # 8 q-tiles per batch
SCALE = 1.0 / float(np.sqrt(H))
NWARM = 10  # PE warm-up matmuls during the DMA fill

f32 = mybir.dt.float32
f16 = mybir.dt.float16

_CACHED = {}


def _plan_chunks(k_pad):
    """Split the doc axis into PSUM-resident chunks: (offset_lo, offset_hi,
    tag, bufs).

    Expected path (k_pad <= 2560): chunk A [128,1024] double-buffered (4
    banks) + chunk B [128, k_pad-1024 <= 1536] single-buffered (<=3
    banks) + 1 bank for the column-sum accumulator = 8.  Generic
    fallback for larger k_pad: 1024-wide chunks cycling three
    single-buffered tags."""
    if k_pad <= 1024:
        return [(0, k_pad, "A", 2)]
    if k_pad <= 2560:
        return [(0, 1024, "A", 2), (1024, k_pad, "B", 1)]
    chunks = []
    off = 0
    i = 0
    while off < k_pad:
        w = min(1024, k_pad - off)
        chunks.append((off, off + w, "BCD"[i % 3], 1))
        off += w
        i += 1
    return chunks


def _build(k_pad):
    chunks = _plan_chunks(k_pad)

    nc = bacc.Bacc("TRN2", target_bir_lowering=False, debug=False)

    qtd = nc.dram_tensor("qt", [BPC, H, QL], f16, kind="ExternalInput")
    dtd = nc.dram_tensor("dt", [BPC, H, k_pad], f16, kind="ExternalInput")
    sd = nc.dram_tensor("s", [BPC, QL, k_pad], f16, kind="ExternalInput")
    ckd = nc.dram_tensor("ck", [BPC, 1], f32, kind="ExternalInput")
    outd = nc.dram_tensor("o", [1, BPC], f32, kind="ExternalOutput")

    with tile.TileContext(nc) as tc:
        with (
            tc.tile_pool(name="const", bufs=1) as const,
            tc.tile_pool(name="qtp", bufs=2) as qtp,
            tc.tile_pool(name="dtp", bufs=2) as dtp,
            tc.tile_pool(name="simp", bufs=5) as simp,
            tc.tile_pool(name="ep", bufs=4) as ep,
            tc.tile_pool(name="pp", bufs=4) as pp,
            tc.tile_pool(name="small", bufs=4) as small,
            tc.tile_pool(name="bsm", bufs=2) as bsm,
            tc.tile_pool(name="outp", bufs=1) as outp,
            tc.tile_pool(name="ps", bufs=1, space="PSUM") as psp,
            tc.tile_pool(name="pacc", bufs=1, space="PSUM") as pacc,
        ):
            # all-zero tile for the fill-phase HAM warm-up matmuls
            # (memset FIRST: it is the only thing the warm-ups wait on)
            wz = const.tile([128, 512], f16, tag="warmz", name="warmz")
            nc.vector.memset(wz, 0.0)
            # stationary w-tiles for the column-sum matmuls: col 0 = 1/den,
            # cols 1..127 stay zero forever (three rotating, since the
            # column-sums trail the scores pipeline by DEFER=2 steps)
            r128s = []
            for ri in range(3):
                r = const.tile([128, 128], f16, tag=f"r128_{ri}", name=f"r128_{ri}")
                nc.vector.memset(r, 0.0)
                r128s.append(r)

            outsb = outp.tile([1, BPC], f32, tag="outsb")

            import concourse.bass as _bass

            # column-sum segments: 256-wide (vs 512) costs no PE streaming
            # time (warm back-to-back gap ~= N/2.4 + NX) but halves the
            # final batch-epilogue reduce on the tail critical path
            CSEG = 256
            ncol = (k_pad + CSEG - 1) // CSEG
            nchunks = len(chunks)
            firstw = chunks[0][1]  # end of the first chunk
            state = {}

            # pad counts for BOTH batches in one broadcast DMA during the
            # fill (partition-broadcast, BPC values per partition row) so
            # batch 1's prefetch never needs the slow 128-descriptor
            # broadcast mid-kernel.
            cntk2 = bsm.tile([128, BPC], f32, tag="cntk", name="cntk")
            ck_ap = ckd.ap()[:, :]
            ck_bcast = _bass.AP(
                tensor=ck_ap.tensor,
                offset=ck_ap.offset,
                ap=[[0, 128], [1, BPC]],
            )

            def setup_batch(b, first):
                dt = dtp.tile([128, k_pad], f16, tag="dt", name=f"dt{b}")
                qt = qtp.tile([128, QL], f16, tag="qt", name=f"qt{b}")
                if first:
                    # Only sync/scalar (HWDGE) and gpsimd (SWDGE) can issue
                    # DMAs.  SDMA engines round-robin between queues at
                    # PACKET granularity (a queue with smaller rows is
                    # starved in proportion), and each transfer's
                    # completion semaphore lags its last packet by the
                    # ~1-2us HBM write-receipt.  PSUM-pool dependencies are
                    # whole-tile, so finer dt pieces cannot start the exp
                    # stream any earlier than the whole chunk anyway: the
                    # fill is exactly one ~2KB-row critical transfer per
                    # queue, all finishing together at the measured floor
                    # (first exp ~12.7us):
                    # scalar: dt chunk A, sync: qt, gpsimd: dt chunk B +
                    # the cnt broadcast.  The sim streams queue BEHIND
                    # these, so they never steal fill bandwidth.
                    nc.scalar.dma_start(
                        dt[:, :firstw], dtd.ap()[b][:, :firstw]
                    )
                    nc.sync.dma_start(qt, qtd.ap()[b])
                    if firstw < k_pad:
                        nc.gpsimd.dma_start(dt[:, firstw:], dtd.ap()[b][:, firstw:])
                    nc.gpsimd.dma_start(cntk2, ck_bcast)
                else:
                    # batch 1's prefetch (emitted mid-batch-0 so its
                    # transfers land between the sim streams, which the
                    # 4-deep sim prefetch absorbs): qt + first dt chunk on
                    # sync, dt tail on gpsimd.
                    nc.sync.dma_start(qt, qtd.ap()[b])
                    nc.sync.dma_start(dt[:, :firstw], dtd.ap()[b][:, :firstw])
                    if firstw < k_pad:
                        nc.gpsimd.dma_start(dt[:, firstw:], dtd.ap()[b][:, firstw:])
                state[b] = (qt, dt)

            setup_batch(0, True)

            # HAM warm-up: the PE sits idle for the ~4.5us DMA fill, which
            # leaves the clock gate at 1.2GHz into the first q-tiles.
            # Dummy matmuls on the all-zero tile (gated only on its
            # memset) keep the PE busy through the fill so it un-throttles
            # (~3.4us sustained) before the first real QK; they park on
            # the PE queue ahead of the real matmuls, ending ~when the
            # first dt piece lands.
            lo, hi, tag0, cbufs0 = chunks[0]
            if hi - lo >= 512:
                for wi in range(NWARM):
                    pscd = psp.tile(
                        [128, hi - lo], f32, tag=tag0, bufs=cbufs0,
                        name=f"pscw{wi}",
                    )
                    nc.tensor.matmul(
                        pscd[:, :512],
                        r128s[0],
                        wz,
                        start=True,
                        stop=True,
                        skip_group_check=True,
                    )

            NSTEP = BPC * QT_N
            # column-sums trail the scores pipeline by DEFER steps so the
            # next q-tiles' score matmuls never wait on the multiply chain
            # (sim DMA -> P -> den/recip -> colsum).  DEFER=2 keeps the
            # in-order PE queue from blocking on tile t's chain while tile
            # t+2's scores are due; measured best among DEFER 1/2/3 (the
            # Tile scheduler makes the final queue order either way).
            DEFER = 2
            pendings = []  # deferred column-sums: (b, t, acc, r128, p_t)

            for s in range(NSTEP + DEFER):
                if s < NSTEP:
                    b, t = divmod(s, QT_N)
                    if t == 5 and b + 1 < BPC:
                        setup_batch(b + 1, False)
                    qt, dt = state[b][0], state[b][1]
                    if t == 0:
                        acc_b = pacc.tile(
                            [128, CSEG], f32, tag="acc", name=f"acc{b}"
                        )
                        state[(b, "acc")] = acc_b
                    acc = state[(b, "acc")]

                    sim = simp.tile(
                        [128, k_pad], f16, tag="sim", name=f"sim{b}_{t}"
                    )
                    # alternate the sim streams across the sync HWDGE queue
                    # and the GpSimd SWDGE queue for DMA parallelism (even
                    # tiles behind qt on sync, odd tiles behind dt-B/cnt on
                    # gpsimd).  The first two tiles' sims are split at the
                    # chunk boundary (slice-level SBUF deps) so each P-mult
                    # piece starts as soon as ITS half lands -- this pulls
                    # the tile-0/1 column-sum chains ~1.5us earlier and
                    # shrinks the ramp's pipeline-fill stall.
                    dma_eng = nc.sync if t % 2 == 0 else nc.gpsimd
                    if s <= 1 and firstw < k_pad:
                        dma_eng.dma_start(
                            sim[:, :firstw],
                            sd.ap()[b, t * 128 : (t + 1) * 128, :firstw],
                        )
                        dma_eng.dma_start(
                            sim[:, firstw:],
                            sd.ap()[b, t * 128 : (t + 1) * 128, firstw:],
                        )
                    else:
                        dma_eng.dma_start(
                            sim, sd.ap()[b, t * 128 : (t + 1) * 128, :]
                        )
                    e_t = ep.tile([128, k_pad], f16, tag="E", name=f"e{b}_{t}")
                    den2 = small.tile(
                        [128, 8], f32, tag="den2", name=f"den2_{b}_{t}"
                    )
                    nparts = 0
                    for ci, (lo, hi, tag, cbufs) in enumerate(chunks):
                        psc = psp.tile(
                            [128, hi - lo], f32, tag=tag, bufs=cbufs,
                            name=f"psc{tag}",
                        )
                        for s0 in range(0, hi - lo, 512):
                            s1 = min(s0 + 512, hi - lo)
                            nc.tensor.matmul(
                                psc[:, s0:s1],
                                qt[:, t * 128 : (t + 1) * 128],
                                dt[:, lo + s0 : lo + s1],
                                start=True,
                                stop=True,
                            )
                        nc.scalar.activation(
                            out=e_t[:, lo:hi],
                            in_=psc,
                            func=mybir.ActivationFunctionType.Exp,
                            scale=SCALE,
                            accum_out=den2[:, nparts : nparts + 1],
                        )
                        nparts += 1

                if s == NSTEP and chunks[0][1] - chunks[0][0] >= 512:
                    # keep-warm: the PE idles ~5us between the last QK and
                    # the last column-sums, long enough for the HAM clock
                    # gate to re-throttle to 1.2GHz.  Two dummy matmuls
                    # gated on the last exp / multiply refresh the activity
                    # window mid-gap so the final column-sums run at 2.4GHz.
                    lo, hi, tag, cbufs = chunks[0]
                    for di, mv in enumerate((last_e, last_p)):
                        pscd = psp.tile(
                            [128, hi - lo], f32, tag=tag, bufs=cbufs,
                            name=f"pscwarm{di}",
                        )
                        nc.tensor.matmul(
                            pscd[:, :512],
                            r128s[1],
                            mv[:, :512],
                            start=True,
                            stop=True,
                            skip_group_check=True,
                        )

                # deferred column-sums, deprioritized so the Tile scheduler
                # keeps them behind the next q-tiles' score matmuls on the
                # in-order PE queue: the scores pipeline must never wait on
                # the DVE multiply chain (sim DMA -> P -> den/recip) that
                # feeds a column-sum.
                if len(pendings) > (DEFER - 1 if s < NSTEP else -1):
                    pb, pt, pacc_t, pr128, p_prev = pendings.pop(0)
                    for j in range(ncol):
                        s0 = j * CSEG
                        s1 = min(s0 + CSEG, k_pad)
                        nc.tensor.matmul(
                            pacc_t[:, : s1 - s0],
                            pr128,
                            p_prev[:, s0:s1],
                            start=(pt == 0 and j == 0),
                            stop=(pt == QT_N - 1 and j == ncol - 1),
                            skip_group_check=True,
                        )
                    if pt == QT_N - 1:
                        # batch epilogue: row 0 of acc = sum over q of
                        # P[q,k]/den_q; store each batch's scalar as soon
                        # as it is ready so only the last batch's [1,1]
                        # DMA sits on the tail.
                        nc.vector.reduce_sum(
                            outsb[0:1, pb : pb + 1],
                            pacc_t[0:1, :],
                            axis=mybir.AxisListType.X,
                        )
                        nc.sync.dma_start(
                            outd.ap()[:, pb : pb + 1],
                            outsb[0:1, pb : pb + 1],
                        )

                if s < NSTEP:
                    cntk = cntk2[:, b : b + 1]
                    # P in pieces, each starting right after the matching
                    # exp ACTIVATE.  All but the last piece are emitted
                    # BEFORE the den chain: the den chain waits on the LAST
                    # chunk's accumulator read, and the DVE queue is strict
                    # FIFO, so P work queued behind it would serialize into
                    # the tail after the final ACTIVATE.
                    p_t = pp.tile([128, k_pad], f16, tag="P", name=f"p{b}_{t}")
                    psplits = sorted(set([0, firstw, k_pad]))
                    for pi in range(len(psplits) - 2):
                        w0, w1 = psplits[pi], psplits[pi + 1]
                        nc.vector.tensor_tensor(
                            p_t[:, w0:w1], e_t[:, w0:w1], sim[:, w0:w1],
                            mybir.AluOpType.mult,
                        )
                    dent = small.tile([128, 1], f32, tag="dent", name="dent")
                    if nparts == 2:
                        # denA - cnt overlaps chunk B's ACTIVATE; only the
                        # tiny add + reciprocal remain on the tail chain
                        denta = small.tile(
                            [128, 1], f32, tag="denta", name="denta"
                        )
                        nc.vector.tensor_scalar(
                            denta, den2[:, 0:1], cntk, None,
                            mybir.AluOpType.subtract,
                        )
                        nc.vector.tensor_tensor(
                            dent, denta, den2[:, 1:2], mybir.AluOpType.add
                        )
                    else:
                        den = small.tile([128, 1], f32, tag="den", name="den")
                        nc.vector.reduce_sum(
                            den, den2[:, :nparts], axis=mybir.AxisListType.X
                        )
                        nc.vector.tensor_scalar(
                            dent, den, cntk, None, mybir.AluOpType.subtract
                        )
                    r128 = r128s[s % 3]
                    with nc.allow_low_precision(
                        reason="1/den in fp16 (11-bit mantissa) feeds the PE "
                        "column-sum; ~5e-4 relative, inside the error budget"
                    ):
                        nc.vector.reciprocal(r128[:, 0:1], dent)
                    w0, w1 = psplits[-2], psplits[-1]
                    nc.vector.tensor_tensor(
                        p_t[:, w0:w1], e_t[:, w0:w1], sim[:, w0:w1],
                        mybir.AluOpType.mult,
                    )
                    pendings.append((b, t, acc, r128, p_t))
                    last_e, last_p = e_t, p_t

    nc.compile()
    return nc


def kernel(**inputs: np.ndarray) -> np.ndarray:
    q = np.asarray(inputs["query_input"], dtype=np.float32)
    d = np.asarray(inputs["doc_input"], dtype=np.float32)
    s = np.asarray(inputs["sim_matrix"], dtype=np.float32)
    dm = np.asarray(inputs["doc_mask"]) != 0  # [B, DL]

    keff = dm.sum(axis=1).astype(np.int64)  # [B]
    k_pad = int(min(DL, max(128, math.ceil(int(keff.max()) / 16) * 16)))

    if k_pad not in _CACHED:
        _CACHED[k_pad] = _build(k_pad)
    nc = _CACHED[k_pad]

    qt = np.ascontiguousarray(np.swapaxes(q, 1, 2)).astype(np.float16)
    dtc = np.zeros((B, H, k_pad), dtype=np.float16)
    simc = np.zeros((B, QL, k_pad), dtype=np.float16)
    for b in range(B):
        idx = np.flatnonzero(dm[b])
        ke = idx.size
        dtc[b, :, :ke] = d[b, idx, :].T
        simc[b, :, :ke] = s[b][:, idx]
    ck = (k_pad - keff).astype(np.float32).reshape(B, 1)

    in_maps = []
    for c in range(NCORES):
        lo, hi = c * BPC, (c + 1) * BPC
        in_maps.append(
            {
                "qt": qt[lo:hi],
                "dt": dtc[lo:hi],
                "s": simc[lo:hi],
                "ck": ck[lo:hi],
            }
        )

    out = None
    for attempt in range(3):
        try:
            res = run_bass_kernel_spmd(nc, in_maps, core_ids=list(range(NCORES)))
            # materialize inside the retry: transient device wedges can
            # surface as late as the device->host copy
            out = np.concatenate(
                [
                    np.asarray(res.results[c]["o"]).reshape(BPC)
                    for c in range(NCORES)
                ],
                axis=0,
            )
            break
        except Exception:
            if attempt == 2:
                raise
    return out.reshape(B, 1).astype(np.float32)
